# revision 65
# baseline (speedup 1.0000x reference)
"""Trainium2 Bass kernel for the truncated-spectrum 2D conv (CF2DConv).

Math: out = iDCT_y( irfft_x( mix_per_mode( rfft_x( DCT_y(x) )[:64,:64] ) ) )
All transforms are dense truncated matrices; the whole op is a chain of
matmuls plus a per-mode complex channel mix.

v2: bf16 data path (fp32 PSUM accumulation), DCT-before-FFT ordering in the
forward pass (truncates Y 512->64 before the X transform, cutting PE work
~2x), dense PSUM drains in the inverse pass with host-side final transpose.

Execution: 3 SPMD launches on 8 NeuronCores.
  phase 1  (shard (b, nx-half)): partial forward spectrum per core
  phase 2a (shard a-modes):      per-mode complex mix, R read exactly once
  phase 2b (shard (b, nx-half)): inverse transforms, dense output writes
Host does the (cheap, few-MB) re-shards between launches.
"""
import numpy as np
import ml_dtypes
from contextlib import ExitStack

import concourse.bass as bass
import concourse.mybir as mybir
import concourse.tile as tile
from concourse.bass_utils import run_bass_kernel_spmd

B, NX, NY, DV = 4, 512, 512, 32
KX, KY = 64, 64
NCORES = 8
NXH = NX // 2          # 256 rows per (b, h) core
F32 = mybir.dt.float32
BF16 = mybir.dt.bfloat16
NPBF16 = ml_dtypes.bfloat16


def _split_multiwait(nc):
    """Each 64B engine instruction has ONE sync-wait slot; Tile can attach
    several (e.g. two operands arriving on different DMAHW sem lanes), which
    walrus codegen rejects ("Too many sync wait commands"). Spill excess
    waits (and updates) onto chains of single-wait no-ops on the same
    engine queue."""
    cnt = 0
    for fn in nc.m.functions:
        for blk in fn.blocks:
            insts = list(blk.instructions)
            out = []
            changed = False
            for inst in insts:
                si = inst.sync_info
                if si is not None:
                    waits = list(si.on_wait or [])
                    ups = list(si.on_update or [])
                    if len(waits) > 1:
                        for w in waits[:-1]:
                            cnt += 1
                            out.append(mybir.InstNoOp(
                                name=f"premw{cnt}_{inst.name}",
                                sync_info=mybir.SyncInfo(on_wait=[w],
                                                         on_update=[]),
                                bass_nofuse=True, engine=inst.engine))
                        inst.sync_info = mybir.SyncInfo(
                            on_wait=waits[-1:], on_update=ups)
                        changed = True
                    if len(ups) > 1:
                        inst.sync_info = mybir.SyncInfo(
                            on_wait=list(inst.sync_info.on_wait or []),
                            on_update=ups[:1])
                        out.append(inst)
                        for u in ups[1:]:
                            cnt += 1
                            out.append(mybir.InstNoOp(
                                name=f"postmw{cnt}_{inst.name}",
                                sync_info=mybir.SyncInfo(on_wait=[],
                                                         on_update=[u]),
                                bass_nofuse=True, engine=inst.engine))
                        changed = True
                        continue
                out.append(inst)
            if changed:
                blk.instructions = out
    return nc


# ----------------------------------------------------------------------------
# Host-side constant transform matrices (float64 -> bf16)
# ----------------------------------------------------------------------------


def _copy(nc, idx, out, in_):
    if idx % 2 == 0:
        nc.scalar.copy(out, in_)
    else:
        nc.vector.tensor_copy(out, in_)





def _build_consts():
    ny = np.arange(NY)
    m = np.arange(KY)
    Cy = np.cos(np.pi * (2 * ny[None, :] + 1) * m[:, None] / (2 * NY))
    s = np.full((KY, 1), np.sqrt(2.0 / NY)); s[0, 0] = np.sqrt(1.0 / NY)
    Cy = Cy * s                                     # [KY, NY]

    nx = np.arange(NX)
    a = np.arange(KX)
    ang = 2 * np.pi * a[:, None] * nx[None, :] / NX
    Fre = np.cos(ang) / np.sqrt(NX)                 # [KX, NX]
    Fim = -np.sin(ang) / np.sqrt(NX)

    w = np.full(KX, 2.0); w[0] = 1.0
    Gr = w[None, :] * np.cos(ang.T) / np.sqrt(NX)   # [NX, KX]
    Gi = -w[None, :] * np.sin(ang.T) / np.sqrt(NX)

    # FxT_all [NX, 128]: cols 0:64 = Fre^T, 64:128 = Fim^T
    FxT = np.concatenate([Fre.T, Fim.T], axis=1)    # [512, 128]
    CyT = Cy.T                                      # [512, 64] DCT lhsT
    # G_all [128(alpha), NX]: rows 0:64 = Gr^T, 64:128 = Gi^T
    G = np.concatenate([Gr.T, Gi.T], axis=0)        # [128, 512]
    return (FxT.astype(NPBF16), CyT.astype(NPBF16),
            G.astype(NPBF16), Cy.astype(NPBF16))


_FXT, _CYT, _G, _CY = _build_consts()
_EYE = np.eye(64, dtype=NPBF16)


def _pack_phase1_consts(h):
    cpk = np.zeros((128, 576), dtype=NPBF16)
    for c in range(4):
        cpk[:, c * 64:(c + 1) * 64] = _CYT[c * 128:(c + 1) * 128]
    fxt = _FXT[h * NXH:(h + 1) * NXH]
    for c2 in range(2):
        cpk[:, 256 + c2 * 128:256 + (c2 + 1) * 128] = (
            fxt[c2 * 128:(c2 + 1) * 128])
    cpk[0:64, 512:576] = _EYE
    return cpk


_CPK = [_pack_phase1_consts(0), _pack_phase1_consts(1)]


# ----------------------------------------------------------------------------
# Phase 1: DCT-Y (contract ny, full) then rFFT-X (contract local nx half)
#   in : xt_in [512, 8192]  ny-major view of this core's x shard, bf16
#        cyt   [512, 64]    Cy^T (DCT lhsT)
#        fxt   [256, 128]   FxT rows for this nx-half
#        ident [64, 64]
#   out: xtr   [128, 2048]  [alpha, (j, m)] fp32  (partial: sum over h needed)
# ----------------------------------------------------------------------------
def build_phase1():
    nc = bass.Bass()
    # x pre-packed on host as [128, (gf 16, c 4, 512)]: fine col-group gf's
    # four ny-chunk blocks are contiguous, so DMA arrival order matches the
    # DCT's consumption order.
    xt_in = nc.declare_dram_parameter("xt_in", [128, NY * NXH * DV // 128],
                                      BF16, isOutput=False)
    # packed consts: cols 0:256 cyt (c at c*64), 256:512 fxt (c2 at c2*128),
    # 512:576 identity (rows 0:64)
    cpk = nc.declare_dram_parameter("cpk", [128, 576], BF16, isOutput=False)
    xtr = nc.declare_dram_parameter("xtr", [128, DV * KY], BF16, isOutput=True)

    with ExitStack() as ctx:
        tc = ctx.enter_context(tile.TileContext(nc))
        consts = ctx.enter_context(tc.tile_pool(name="consts", bufs=1))
        xpool = ctx.enter_context(tc.tile_pool(name="xpool", bufs=1))
        tpool = ctx.enter_context(tc.tile_pool(name="tpool", bufs=1))
        ttpool = ctx.enter_context(tc.tile_pool(name="ttpool", bufs=1))
        xtrpool = ctx.enter_context(tc.tile_pool(name="xtrpool", bufs=1))
        psD = ctx.enter_context(tc.tile_pool(name="psD", bufs=3, space="PSUM"))
        psT = ctx.enter_context(tc.tile_pool(name="psT", bufs=2, space="PSUM"))
        psF = ctx.enter_context(tc.tile_pool(name="psF", bufs=1, space="PSUM"))

        cpk_t = consts.tile([128, 576], BF16)
        nc.sync.dma_start(out=cpk_t, in_=cpk[:, :])
        id_t = cpk_t[0:64, 512:576]

        # x tiles [128, 2048]: tile t holds fine group t complete
        xts = []
        for t in range(16):
            xt = xpool.tile([128, 2048], BF16, tag=f"xt{t}", name=f"xt{t}")
            nc.sync.dma_start(out=xt,
                              in_=xt_in[:, t * 2048:(t + 1) * 2048])
            xts.append(xt)

        # T cols are (j 32, nx 256): col = j*256 + nx
        T = tpool.tile([64, NXH * DV], BF16, tag="T", name="T")
        TT = ttpool.tile([128, 2 * DV * KY], BF16, tag="TT", name="TT")
        xtr_s = xtrpool.tile([128, DV * KY], BF16, tag="xtr", name="xtr_s")

        # ---- stage DCT-Y: T[m 64, (j, nx)] = Cy @ x ----
        for gf in range(16):                         # 512-col fine groups
            ps = psD.tile([64, 512], F32, tag="dct", name=f"dct{gf}")
            for c in range(4):
                nc.tensor.matmul(
                    ps, cpk_t[:, c * KY:(c + 1) * KY],
                    xts[gf][:, c * 512:(c + 1) * 512],
                    start=(c == 0), stop=(c == 3))
            _copy(nc, gf, T[:, gf * 512:(gf + 1) * 512], ps)

        # ---- per j-octet: transposes then FFT partials then output DMA,
        # pipelined against later DCT groups ----
        for jg in range(4):
            pTs = [psT.tile([128, 512], BF16, tag=f"pT{c2}",
                            name=f"pT{c2}_{jg}") for c2 in range(2)]
            for jj in range(8):
                j = jg * 8 + jj
                for c2 in range(2):
                    nc.tensor.transpose(
                        pTs[c2][:, jj * KY:(jj + 1) * KY],
                        T[:, j * NXH + c2 * 128:j * NXH + (c2 + 1) * 128],
                        id_t)
            for c2 in range(2):
                _copy(nc, jg + c2, TT[:, c2 * 2048 + jg * 512:
                                      c2 * 2048 + (jg + 1) * 512], pTs[c2])

            # ---- stage rFFT-X (contract nx): xtr[alpha, (j, m)] ----
            ps = psF.tile([128, 512], F32, tag="fft", name=f"fft{jg}")
            for c2 in range(2):
                nc.tensor.matmul(
                    ps, cpk_t[:, 256 + c2 * 128:256 + (c2 + 1) * 128],
                    TT[:, c2 * 2048 + jg * 512:c2 * 2048 + (jg + 1) * 512],
                    start=(c2 == 0), stop=(c2 == 1))
            _copy(nc, jg, xtr_s[:, jg * 512:(jg + 1) * 512], ps)
            nc.sync.dma_start(out=xtr[:, jg * 512:(jg + 1) * 512],
                              in_=xtr_s[:, jg * 512:(jg + 1) * 512])
    return _split_multiwait(nc)


# ----------------------------------------------------------------------------
# Phase 2a: per-mode complex channel mix, sharded over a (8 a-values per core)
#   in : w2   [128, 256*64]  [(rr/ri, j), (g, i32)]  R slice, bf16
#        x2   [128, 256*8]   [(p, j), (g, q, b)] spectrum, bf16
#   out: y    [64, 8*64*4]   [(u, i), (g, q, b)] fp32
# ----------------------------------------------------------------------------
def build_phase2a():
    NMODE = (KX // NCORES) * KY                      # 512 modes per core
    NG = NMODE // 2                                  # 256 mode-pair groups
    nc = bass.Bass()
    w2 = nc.declare_dram_parameter("w2", [128, NG * 64], BF16, isOutput=False)
    x2 = nc.declare_dram_parameter("x2", [128, NG * 8], BF16, isOutput=False)
    y = nc.declare_dram_parameter("y", [64, NMODE * B], BF16, isOutput=True)

    with ExitStack() as ctx:
        tc = ctx.enter_context(tile.TileContext(nc))
        consts = ctx.enter_context(tc.tile_pool(name="consts", bufs=1))
        outpool = ctx.enter_context(tc.tile_pool(name="outpool", bufs=1))
        psY = ctx.enter_context(tc.tile_pool(name="psY", bufs=4, space="PSUM"))

        w_ts = [consts.tile([128, 2048], BF16, tag=f"w{c}", name=f"w{c}")
                for c in range(8)]
        x_ts = [consts.tile([128, NG * 4], BF16, tag=f"x{c}", name=f"x{c}")
                for c in range(2)]
        nc.sync.dma_start(out=w_ts[0], in_=w2[:, 0:2048])
        for c in range(2):
            nc.sync.dma_start(out=x_ts[c],
                              in_=x2[:, c * NG * 4:(c + 1) * NG * 4])
        for c in range(1, 8):
            nc.sync.dma_start(out=w_ts[c], in_=w2[:, c * 2048:(c + 1) * 2048])
        y_ts = [outpool.tile([64, 512], BF16, tag=f"y{bk}", name=f"y{bk}")
                for bk in range(4)]

        for bk in range(4):                          # 64 groups per psum bank
            pY = psY.tile([64, 512], F32)
            for gg in range(64):
                g = bk * 64 + gg
                nc.tensor.matmul(pY[:, gg * 8:(gg + 1) * 8],
                                 w_ts[g // 32][:, (g % 32) * 64:
                                               (g % 32 + 1) * 64],
                                 x_ts[g // 128][:, (g % 128) * 8:
                                                (g % 128 + 1) * 8],
                                 start=True, stop=True)
            _copy(nc, bk, y_ts[bk], pY)
            nc.sync.dma_start(out=y[:, bk * 512:(bk + 1) * 512], in_=y_ts[bk])
    return _split_multiwait(nc)


# ----------------------------------------------------------------------------
# Phase 2b: inverse transforms per (b, nx-half)
#   in : yb  [128, 2048]  [(q, a), (i, m)] bf16
#        gh  [128, 256]   G rows alpha, cols nx-local, bf16
#        cym [64, 512]    Cy [m, ny] bf16
#   out: oh2 [256, 16384] rows nx-local, cols (i, ny) bf16
# ----------------------------------------------------------------------------
def build_phase2b():
    nc = bass.Bass()
    # packed: cols 0:2048 yb [(q,a),(i,m)], 2048:2304 gh, 2304:2816 cym(r0:64)
    ypk = nc.declare_dram_parameter("ypk", [128, 2816], BF16, isOutput=False)
    oh2 = nc.declare_dram_parameter("oh2", [NXH, DV * NY], BF16, isOutput=True)

    with ExitStack() as ctx:
        tc = ctx.enter_context(tile.TileContext(nc))
        consts = ctx.enter_context(tc.tile_pool(name="consts", bufs=1))
        yrpool = ctx.enter_context(tc.tile_pool(name="yrpool", bufs=1))
        opool = ctx.enter_context(tc.tile_pool(name="opool", bufs=3))
        psD = ctx.enter_context(tc.tile_pool(name="psD", bufs=2, space="PSUM"))
        psE = ctx.enter_context(tc.tile_pool(name="psE", bufs=3, space="PSUM"))

        ypk_t = consts.tile([128, 2816], BF16)
        nc.sync.dma_start(out=ypk_t[:, 2048:2816], in_=ypk[:, 2048:2816])
        nc.sync.dma_start(out=ypk_t[:, 0:1024], in_=ypk[:, 0:1024])
        nc.sync.dma_start(out=ypk_t[:, 1024:2048], in_=ypk[:, 1024:2048])
        gh_t = ypk_t[:, 2048:2304]
        # cym duplicated in both partition halves so stage E can read the
        # half matching its lhsT base partition
        cym_h = [ypk_t[0:64, 2304:2816], ypk_t[64:128, 2304:2816]]

        # stage D: yr [128 = (i-pair, m), nx 256] = yb[:, i-pair]^T @ gh —
        # two i-channels share the partition dim so drains halve in columns
        YRs = [yrpool.tile([128, NXH], BF16, tag=f"YR{ip}", bufs=1,
                           name=f"YR{ip}") for ip in range(16)]
        for ip in range(DV // 2):
            pD = psD.tile([128, NXH], F32)
            nc.tensor.matmul(pD, ypk_t[:, ip * 128:(ip + 1) * 128], gh_t,
                             start=True, stop=True)
            _copy(nc, ip, YRs[ip], pD)

        # stage E: out[nx 128, ny 512] per (i, kc); dense drains into
        # [nx, (i, ny)] tiles (host re-transposes to [nx, ny, i] for free).
        for kc in range(2):
            for ig in range(4):                      # 8 i's per output tile
                Oh = opool.tile([128, 8 * NY], BF16, tag="Oh",
                                name=f"Oh{kc}_{ig}")
                for ii2 in range(4):                 # i-pair per psum tile
                    pE = psE.tile([128, 2 * NY], F32)
                    for ii in (ii2 * 2, ii2 * 2 + 1):
                        i = ig * 8 + ii
                        nc.tensor.matmul(
                            pE[:, (ii % 2) * NY:(ii % 2 + 1) * NY],
                            YRs[i // 2][(i % 2) * 64:(i % 2) * 64 + 64,
                                        kc * 128:(kc + 1) * 128],
                            cym_h[i % 2], start=True, stop=True)
                    _copy(nc, ii2 + kc, Oh[:, ii2 * 2 * NY:(ii2 + 1) * 2 * NY],
                          pE)
                nc.sync.dma_start(
                    out=oh2[kc * 128:(kc + 1) * 128,
                            ig * 8 * NY:(ig + 1) * 8 * NY],
                    in_=Oh)
    return _split_multiwait(nc)


_NC_CACHE = {}
LAST_EXEC_NS = []


def _get(name):
    if name not in _NC_CACHE:
        _NC_CACHE[name] = {"p1": build_phase1, "p2a": build_phase2a,
                           "p2b": build_phase2b}[name]()
    return _NC_CACHE[name]


def kernel(x, R_real, R_imag):
    x = np.ascontiguousarray(x, dtype=np.float32)
    AL = KX // NCORES

    # ---------------- phase 1 ----------------
    in1 = []
    for c in range(NCORES):
        b, h = c // 2, c % 2
        xh = x[b, h * NXH:(h + 1) * NXH]              # [256, 512, 32]
        xt = xh.transpose(1, 2, 0).reshape(NY, NXH * DV)   # [ny, (j, nx)]
        # pack [(c 4, p 128) ny, (gf 16, 512) col] -> [p, (gf, c, 512)]
        xp = (xt.reshape(4, 128, 16, 512).transpose(1, 2, 0, 3)
              .reshape(128, NY * NXH * DV // 128))
        in1.append({
            "xt_in": np.ascontiguousarray(xp).astype(NPBF16),
            "cpk": _CPK[h],
        })
    LAST_EXEC_NS.clear()
    r1 = run_bass_kernel_spmd(_get("p1"), in1, list(range(NCORES)))
    LAST_EXEC_NS.append(r1.exec_time_ns)
    # partials [alpha, j, m] per (b, h); sum halves -> spect [B, 128, 32, 64]
    parts = [r1.results[c]["xtr"].astype(np.float32).reshape(128, DV, KY)
             for c in range(NCORES)]
    spect = np.stack([parts[2 * b] + parts[2 * b + 1] for b in range(B)])

    # ---------------- phase 2a ----------------
    NMODE = AL * KY
    NG = NMODE // 2
    in2 = []
    for s in range(NCORES):
        a_sl = slice(s * AL, (s + 1) * AL)
        # [j, i, mode] slices of R (mode = a_l*64 + m)
        Rr_t = R_real[:, :, a_sl, :].transpose(1, 0, 2, 3).reshape(DV, DV, NMODE)
        Ri_t = R_imag[:, :, a_sl, :].transpose(1, 0, 2, 3).reshape(DV, DV, NMODE)
        W2 = np.zeros((128, NG, 64), dtype=np.float32)
        # spect [B, alpha, j, m] -> xr/xi [j, mode, b]
        xr = spect[:, a_sl, :, :].transpose(2, 1, 3, 0).reshape(DV, NMODE, B)
        xi = (spect[:, 64 + s * AL:64 + (s + 1) * AL, :, :]
              .transpose(2, 1, 3, 0).reshape(DV, NMODE, B))
        X2 = np.empty((128, NG, 2, B), dtype=np.float32)
        for u in range(2):
            r0, r1_, r2_ = u * 64, u * 64 + 32, u * 64 + 64
            W2[r0:r1_, :, u * 32:(u + 1) * 32] = (
                Rr_t[:, :, u::2].transpose(0, 2, 1))
            W2[r1_:r2_, :, u * 32:(u + 1) * 32] = (
                Ri_t[:, :, u::2].transpose(0, 2, 1))
            X2[r0:r1_, :, 0, :] = xr[:, u::2, :]
            X2[r1_:r2_, :, 0, :] = -xi[:, u::2, :]
            X2[r0:r1_, :, 1, :] = xi[:, u::2, :]
            X2[r1_:r2_, :, 1, :] = xr[:, u::2, :]
        in2.append({"w2": W2.reshape(128, NG * 64).astype(NPBF16),
                    "x2": X2.reshape(128, NG * 8).astype(NPBF16)})
    r2 = run_bass_kernel_spmd(_get("p2a"), in2, list(range(NCORES)))
    LAST_EXEC_NS.append(r2.exec_time_ns)
    # y core result [64=(u,i), (g, q, b)] -> [q, i, a_l, m, b] per core
    ys = []
    for s in range(NCORES):
        t = r2.results[s]["y"].reshape(2, DV, NG, 2, B)       # [u, i, g, q, b]
        t = t.transpose(3, 1, 2, 0, 4).reshape(2, DV, NMODE, B)
        ys.append(t.reshape(2, DV, AL, KY, B))
    yfull = np.stack(ys)                                       # [s, q, i, a_l, m, b]
    yfull = yfull.transpose(1, 2, 0, 3, 4, 5).reshape(2, DV, KX, KY, B)

    # ---------------- phase 2b ----------------
    in3 = []
    for c in range(NCORES):
        b, h = c // 2, c % 2
        ypk = np.zeros((128, 2816), dtype=NPBF16)
        # yb [(q, a), (i, m)]
        ybc = yfull[:, :, :, :, b].transpose(0, 2, 1, 3).reshape(128, DV * KY)
        ypk[:, 0:2048] = ybc.astype(NPBF16)
        ypk[:, 2048:2304] = _G[:, h * NXH:(h + 1) * NXH]
        ypk[0:64, 2304:2816] = _CY
        ypk[64:128, 2304:2816] = _CY
        in3.append({"ypk": ypk})
    r3 = run_bass_kernel_spmd(_get("p2b"), in3, list(range(NCORES)))
    LAST_EXEC_NS.append(r3.exec_time_ns)

    out = np.empty((B, NX, NY, DV), dtype=np.float32)
    for c in range(NCORES):
        b, h = c // 2, c % 2
        oh2 = r3.results[c]["oh2"].reshape(NXH, DV, NY)
        out[b, h * NXH:(h + 1) * NXH] = (
            oh2.transpose(0, 2, 1).astype(np.float32))
    return out


# revision 68
# speedup vs baseline: 1.0036x; 1.0036x over previous
"""Trainium2 Bass kernel for the truncated-spectrum 2D conv (CF2DConv).

Math: out = iDCT_y( irfft_x( mix_per_mode( rfft_x( DCT_y(x) )[:64,:64] ) ) )
All transforms are dense truncated matrices; the whole op is a chain of
matmuls plus a per-mode complex channel mix.

v2: bf16 data path (fp32 PSUM accumulation), DCT-before-FFT ordering in the
forward pass (truncates Y 512->64 before the X transform, cutting PE work
~2x), dense PSUM drains in the inverse pass with host-side final transpose.

Execution: 3 SPMD launches on 8 NeuronCores.
  phase 1  (shard (b, nx-half)): partial forward spectrum per core
  phase 2a (shard a-modes):      per-mode complex mix, R read exactly once
  phase 2b (shard (b, nx-half)): inverse transforms, dense output writes
Host does the (cheap, few-MB) re-shards between launches.
"""
import numpy as np
import ml_dtypes
from contextlib import ExitStack

import concourse.bass as bass
import concourse.mybir as mybir
import concourse.tile as tile
from concourse.bass_utils import run_bass_kernel_spmd

B, NX, NY, DV = 4, 512, 512, 32
KX, KY = 64, 64
NCORES = 8
NXH = NX // 2          # 256 rows per (b, h) core
F32 = mybir.dt.float32
BF16 = mybir.dt.bfloat16
NPBF16 = ml_dtypes.bfloat16


def _split_multiwait(nc):
    """Each 64B engine instruction has ONE sync-wait slot; Tile can attach
    several (e.g. two operands arriving on different DMAHW sem lanes), which
    walrus codegen rejects ("Too many sync wait commands"). Spill excess
    waits (and updates) onto chains of single-wait no-ops on the same
    engine queue."""
    cnt = 0
    for fn in nc.m.functions:
        for blk in fn.blocks:
            insts = list(blk.instructions)
            out = []
            changed = False
            for inst in insts:
                si = inst.sync_info
                if si is not None:
                    waits = list(si.on_wait or [])
                    ups = list(si.on_update or [])
                    if len(waits) > 1:
                        for w in waits[:-1]:
                            cnt += 1
                            out.append(mybir.InstNoOp(
                                name=f"premw{cnt}_{inst.name}",
                                sync_info=mybir.SyncInfo(on_wait=[w],
                                                         on_update=[]),
                                bass_nofuse=True, engine=inst.engine))
                        inst.sync_info = mybir.SyncInfo(
                            on_wait=waits[-1:], on_update=ups)
                        changed = True
                    if len(ups) > 1:
                        inst.sync_info = mybir.SyncInfo(
                            on_wait=list(inst.sync_info.on_wait or []),
                            on_update=ups[:1])
                        out.append(inst)
                        for u in ups[1:]:
                            cnt += 1
                            out.append(mybir.InstNoOp(
                                name=f"postmw{cnt}_{inst.name}",
                                sync_info=mybir.SyncInfo(on_wait=[],
                                                         on_update=[u]),
                                bass_nofuse=True, engine=inst.engine))
                        changed = True
                        continue
                out.append(inst)
            if changed:
                blk.instructions = out
    return nc


# ----------------------------------------------------------------------------
# Host-side constant transform matrices (float64 -> bf16)
# ----------------------------------------------------------------------------


def _copy(nc, idx, out, in_):
    if idx % 2 == 0:
        nc.scalar.copy(out, in_)
    else:
        nc.vector.tensor_copy(out, in_)





def _build_consts():
    ny = np.arange(NY)
    m = np.arange(KY)
    Cy = np.cos(np.pi * (2 * ny[None, :] + 1) * m[:, None] / (2 * NY))
    s = np.full((KY, 1), np.sqrt(2.0 / NY)); s[0, 0] = np.sqrt(1.0 / NY)
    Cy = Cy * s                                     # [KY, NY]

    nx = np.arange(NX)
    a = np.arange(KX)
    ang = 2 * np.pi * a[:, None] * nx[None, :] / NX
    Fre = np.cos(ang) / np.sqrt(NX)                 # [KX, NX]
    Fim = -np.sin(ang) / np.sqrt(NX)

    w = np.full(KX, 2.0); w[0] = 1.0
    Gr = w[None, :] * np.cos(ang.T) / np.sqrt(NX)   # [NX, KX]
    Gi = -w[None, :] * np.sin(ang.T) / np.sqrt(NX)

    # FxT_all [NX, 128]: cols 0:64 = Fre^T, 64:128 = Fim^T
    FxT = np.concatenate([Fre.T, Fim.T], axis=1)    # [512, 128]
    CyT = Cy.T                                      # [512, 64] DCT lhsT
    # G_all [128(alpha), NX]: rows 0:64 = Gr^T, 64:128 = Gi^T
    G = np.concatenate([Gr.T, Gi.T], axis=0)        # [128, 512]
    return (FxT.astype(NPBF16), CyT.astype(NPBF16),
            G.astype(NPBF16), Cy.astype(NPBF16))


_FXT, _CYT, _G, _CY = _build_consts()
_EYE = np.eye(64, dtype=NPBF16)


def _pack_phase1_consts(h):
    cpk = np.zeros((128, 576), dtype=NPBF16)
    for c in range(4):
        cpk[:, c * 64:(c + 1) * 64] = _CYT[c * 128:(c + 1) * 128]
    fxt = _FXT[h * NXH:(h + 1) * NXH]
    for c2 in range(2):
        cpk[:, 256 + c2 * 128:256 + (c2 + 1) * 128] = (
            fxt[c2 * 128:(c2 + 1) * 128])
    cpk[0:64, 512:576] = _EYE
    return cpk


_CPK = [_pack_phase1_consts(0), _pack_phase1_consts(1)]


# ----------------------------------------------------------------------------
# Phase 1: DCT-Y (contract ny, full) then rFFT-X (contract local nx half)
#   in : xt_in [512, 8192]  ny-major view of this core's x shard, bf16
#        cyt   [512, 64]    Cy^T (DCT lhsT)
#        fxt   [256, 128]   FxT rows for this nx-half
#        ident [64, 64]
#   out: xtr   [128, 2048]  [alpha, (j, m)] fp32  (partial: sum over h needed)
# ----------------------------------------------------------------------------
def build_phase1():
    nc = bass.Bass()
    # x pre-packed on host as [128, (gf 16, c 4, 512)]: fine col-group gf's
    # four ny-chunk blocks are contiguous, so DMA arrival order matches the
    # DCT's consumption order.
    xt_in = nc.declare_dram_parameter("xt_in", [128, NY * NXH * DV // 128],
                                      BF16, isOutput=False)
    # packed consts: cols 0:256 cyt (c at c*64), 256:512 fxt (c2 at c2*128),
    # 512:576 identity (rows 0:64)
    cpk = nc.declare_dram_parameter("cpk", [128, 576], BF16, isOutput=False)
    xtr = nc.declare_dram_parameter("xtr", [128, DV * KY], BF16, isOutput=True)

    with ExitStack() as ctx:
        tc = ctx.enter_context(tile.TileContext(nc))
        consts = ctx.enter_context(tc.tile_pool(name="consts", bufs=1))
        xpool = ctx.enter_context(tc.tile_pool(name="xpool", bufs=1))
        tpool = ctx.enter_context(tc.tile_pool(name="tpool", bufs=1))
        ttpool = ctx.enter_context(tc.tile_pool(name="ttpool", bufs=1))
        xtrpool = ctx.enter_context(tc.tile_pool(name="xtrpool", bufs=1))
        psD = ctx.enter_context(tc.tile_pool(name="psD", bufs=3, space="PSUM"))
        psT = ctx.enter_context(tc.tile_pool(name="psT", bufs=2, space="PSUM"))
        psF = ctx.enter_context(tc.tile_pool(name="psF", bufs=1, space="PSUM"))

        cpk_t = consts.tile([128, 576], BF16)
        nc.sync.dma_start(out=cpk_t, in_=cpk[:, :])
        id_t = cpk_t[0:64, 512:576]

        # x tiles [128, 2048]: tile t holds fine group t complete
        xts = []
        for t in range(16):
            xt = xpool.tile([128, 2048], BF16, tag=f"xt{t}", name=f"xt{t}")
            nc.sync.dma_start(out=xt,
                              in_=xt_in[:, t * 2048:(t + 1) * 2048])
            xts.append(xt)

        # T cols are (j 32, nx 256): col = j*256 + nx
        T = tpool.tile([64, NXH * DV], BF16, tag="T", name="T")
        TT = ttpool.tile([128, 2 * DV * KY], BF16, tag="TT", name="TT")
        xtr_s = xtrpool.tile([128, DV * KY], BF16, tag="xtr", name="xtr_s")

        # ---- stage DCT-Y: T[m 64, (j, nx)] = Cy @ x ----
        for gf in range(16):                         # 512-col fine groups
            ps = psD.tile([64, 512], F32, tag="dct", name=f"dct{gf}")
            for c in range(4):
                nc.tensor.matmul(
                    ps, cpk_t[:, c * KY:(c + 1) * KY],
                    xts[gf][:, c * 512:(c + 1) * 512],
                    start=(c == 0), stop=(c == 3))
            _copy(nc, gf, T[:, gf * 512:(gf + 1) * 512], ps)

        # ---- per j-octet: transposes then FFT partials then output DMA,
        # pipelined against later DCT groups ----
        for jg in range(4):
            pTs = [psT.tile([128, 512], BF16, tag=f"pT{c2}",
                            name=f"pT{c2}_{jg}") for c2 in range(2)]
            for jj in range(8):
                j = jg * 8 + jj
                for c2 in range(2):
                    nc.tensor.transpose(
                        pTs[c2][:, jj * KY:(jj + 1) * KY],
                        T[:, j * NXH + c2 * 128:j * NXH + (c2 + 1) * 128],
                        id_t)
            for c2 in range(2):
                _copy(nc, jg + c2, TT[:, c2 * 2048 + jg * 512:
                                      c2 * 2048 + (jg + 1) * 512], pTs[c2])

            # ---- stage rFFT-X (contract nx): xtr[alpha, (j, m)] ----
            ps = psF.tile([128, 512], F32, tag="fft", name=f"fft{jg}")
            for c2 in range(2):
                nc.tensor.matmul(
                    ps, cpk_t[:, 256 + c2 * 128:256 + (c2 + 1) * 128],
                    TT[:, c2 * 2048 + jg * 512:c2 * 2048 + (jg + 1) * 512],
                    start=(c2 == 0), stop=(c2 == 1))
            _copy(nc, jg, xtr_s[:, jg * 512:(jg + 1) * 512], ps)
            nc.sync.dma_start(out=xtr[:, jg * 512:(jg + 1) * 512],
                              in_=xtr_s[:, jg * 512:(jg + 1) * 512])
    return _split_multiwait(nc)


# ----------------------------------------------------------------------------
# Phase 2a: per-mode complex channel mix, sharded over a (8 a-values per core)
#   in : w2   [128, 256*64]  [(rr/ri, j), (g, i32)]  R slice, bf16
#        x2   [128, 256*8]   [(p, j), (g, q, b)] spectrum, bf16
#   out: y    [64, 8*64*4]   [(u, i), (g, q, b)] fp32
# ----------------------------------------------------------------------------
def build_phase2a():
    NMODE = (KX // NCORES) * KY                      # 512 modes per core
    NG = NMODE // 2                                  # 256 mode-pair groups
    nc = bass.Bass()
    w2 = nc.declare_dram_parameter("w2", [128, NG * 64], BF16, isOutput=False)
    x2 = nc.declare_dram_parameter("x2", [128, NG * 8], BF16, isOutput=False)
    # y rows: 0:64 even pair-group (u,i), 64:128 odd; cols (dg, 16)
    y = nc.declare_dram_parameter("y", [128, NMODE * B], BF16, isOutput=True)

    with ExitStack() as ctx:
        tc = ctx.enter_context(tile.TileContext(nc))
        consts = ctx.enter_context(tc.tile_pool(name="consts", bufs=1))
        outpool = ctx.enter_context(tc.tile_pool(name="outpool", bufs=1))
        psY = ctx.enter_context(tc.tile_pool(name="psY", bufs=4, space="PSUM"))

        w_ts = [consts.tile([128, 2048], BF16, tag=f"w{c}", name=f"w{c}")
                for c in range(8)]
        x_ts = [consts.tile([128, NG * 4], BF16, tag=f"x{c}", name=f"x{c}")
                for c in range(2)]
        nc.sync.dma_start(out=w_ts[0], in_=w2[:, 0:2048])
        for c in range(2):
            nc.sync.dma_start(out=x_ts[c],
                              in_=x2[:, c * NG * 4:(c + 1) * NG * 4])
        for c in range(1, 8):
            nc.sync.dma_start(out=w_ts[c], in_=w2[:, c * 2048:(c + 1) * 2048])
        y_ts = [outpool.tile([128, 512], BF16, tag=f"y{bk}", name=f"y{bk}")
                for bk in range(4)]

        # 2 mode-pair groups per LDWEIGHTS: lhsT [128, 128] = [W_even|W_odd],
        # rhs [128, 16] = [x_even|x_odd]; useful output lives in the diagonal
        # quadrants (rows 0:64 x cols 0:8, rows 64:128 x cols 8:16).
        for bk in range(4):                          # 32 double-groups/bank
            pY = psY.tile([128, 512], F32)
            for dgg in range(32):
                dg = bk * 32 + dgg
                nc.tensor.matmul(pY[:, dgg * 16:(dgg + 1) * 16],
                                 w_ts[dg // 16][:, (dg % 16) * 128:
                                                (dg % 16 + 1) * 128],
                                 x_ts[dg // 64][:, (dg % 64) * 16:
                                                (dg % 64 + 1) * 16],
                                 start=True, stop=True)
            _copy(nc, bk, y_ts[bk], pY)
            nc.sync.dma_start(out=y[:, bk * 512:(bk + 1) * 512], in_=y_ts[bk])
    return _split_multiwait(nc)


# ----------------------------------------------------------------------------
# Phase 2b: inverse transforms per (b, nx-half)
#   in : yb  [128, 2048]  [(q, a), (i, m)] bf16
#        gh  [128, 256]   G rows alpha, cols nx-local, bf16
#        cym [64, 512]    Cy [m, ny] bf16
#   out: oh2 [256, 16384] rows nx-local, cols (i, ny) bf16
# ----------------------------------------------------------------------------
def build_phase2b():
    nc = bass.Bass()
    # packed: cols 0:2048 yb [(q,a),(i,m)], 2048:2304 gh, 2304:2816 cym(r0:64)
    ypk = nc.declare_dram_parameter("ypk", [128, 2816], BF16, isOutput=False)
    oh2 = nc.declare_dram_parameter("oh2", [NXH, DV * NY], BF16, isOutput=True)

    with ExitStack() as ctx:
        tc = ctx.enter_context(tile.TileContext(nc))
        consts = ctx.enter_context(tc.tile_pool(name="consts", bufs=1))
        yrpool = ctx.enter_context(tc.tile_pool(name="yrpool", bufs=1))
        opool = ctx.enter_context(tc.tile_pool(name="opool", bufs=3))
        psD = ctx.enter_context(tc.tile_pool(name="psD", bufs=2, space="PSUM"))
        psE = ctx.enter_context(tc.tile_pool(name="psE", bufs=3, space="PSUM"))

        ypk_t = consts.tile([128, 2816], BF16)
        nc.sync.dma_start(out=ypk_t[:, 2048:2816], in_=ypk[:, 2048:2816])
        nc.sync.dma_start(out=ypk_t[:, 0:1024], in_=ypk[:, 0:1024])
        nc.sync.dma_start(out=ypk_t[:, 1024:2048], in_=ypk[:, 1024:2048])
        gh_t = ypk_t[:, 2048:2304]
        # cym duplicated in both partition halves so stage E can read the
        # half matching its lhsT base partition
        cym_h = [ypk_t[0:64, 2304:2816], ypk_t[64:128, 2304:2816]]

        # stage D: yr [128 = (i-pair, m), nx 256] = yb[:, i-pair]^T @ gh —
        # two i-channels share the partition dim so drains halve in columns
        YRs = [yrpool.tile([128, NXH], BF16, tag=f"YR{ip}", bufs=1,
                           name=f"YR{ip}") for ip in range(16)]
        for ip in range(DV // 2):
            pD = psD.tile([128, NXH], F32)
            nc.tensor.matmul(pD, ypk_t[:, ip * 128:(ip + 1) * 128], gh_t,
                             start=True, stop=True)
            _copy(nc, ip, YRs[ip], pD)

        # stage E: out[nx 128, ny 512] per (i, kc); dense drains into
        # [nx, (i, ny)] tiles (host re-transposes to [nx, ny, i] for free).
        for kc in range(2):
            for ig in range(4):                      # 8 i's per output tile
                Oh = opool.tile([128, 8 * NY], BF16, tag="Oh",
                                name=f"Oh{kc}_{ig}")
                for ii2 in range(4):                 # i-pair per psum tile
                    pE = psE.tile([128, 2 * NY], F32)
                    for ii in (ii2 * 2, ii2 * 2 + 1):
                        i = ig * 8 + ii
                        nc.tensor.matmul(
                            pE[:, (ii % 2) * NY:(ii % 2 + 1) * NY],
                            YRs[i // 2][(i % 2) * 64:(i % 2) * 64 + 64,
                                        kc * 128:(kc + 1) * 128],
                            cym_h[i % 2], start=True, stop=True)
                    _copy(nc, ii2 + kc, Oh[:, ii2 * 2 * NY:(ii2 + 1) * 2 * NY],
                          pE)
                nc.sync.dma_start(
                    out=oh2[kc * 128:(kc + 1) * 128,
                            ig * 8 * NY:(ig + 1) * 8 * NY],
                    in_=Oh)
    return _split_multiwait(nc)


_NC_CACHE = {}
LAST_EXEC_NS = []


def _get(name):
    if name not in _NC_CACHE:
        _NC_CACHE[name] = {"p1": build_phase1, "p2a": build_phase2a,
                           "p2b": build_phase2b}[name]()
    return _NC_CACHE[name]


def kernel(x, R_real, R_imag):
    x = np.ascontiguousarray(x, dtype=np.float32)
    AL = KX // NCORES

    # ---------------- phase 1 ----------------
    in1 = []
    for c in range(NCORES):
        b, h = c // 2, c % 2
        xh = x[b, h * NXH:(h + 1) * NXH]              # [256, 512, 32]
        xt = xh.transpose(1, 2, 0).reshape(NY, NXH * DV)   # [ny, (j, nx)]
        # pack [(c 4, p 128) ny, (gf 16, 512) col] -> [p, (gf, c, 512)]
        xp = (xt.reshape(4, 128, 16, 512).transpose(1, 2, 0, 3)
              .reshape(128, NY * NXH * DV // 128))
        in1.append({
            "xt_in": np.ascontiguousarray(xp).astype(NPBF16),
            "cpk": _CPK[h],
        })
    LAST_EXEC_NS.clear()
    r1 = run_bass_kernel_spmd(_get("p1"), in1, list(range(NCORES)))
    LAST_EXEC_NS.append(r1.exec_time_ns)
    # partials [alpha, j, m] per (b, h); sum halves -> spect [B, 128, 32, 64]
    parts = [r1.results[c]["xtr"].astype(np.float32).reshape(128, DV, KY)
             for c in range(NCORES)]
    spect = np.stack([parts[2 * b] + parts[2 * b + 1] for b in range(B)])

    # ---------------- phase 2a ----------------
    NMODE = AL * KY
    NG = NMODE // 2
    in2 = []
    for s in range(NCORES):
        a_sl = slice(s * AL, (s + 1) * AL)
        # [j, i, mode] slices of R (mode = a_l*64 + m)
        Rr_t = R_real[:, :, a_sl, :].transpose(1, 0, 2, 3).reshape(DV, DV, NMODE)
        Ri_t = R_imag[:, :, a_sl, :].transpose(1, 0, 2, 3).reshape(DV, DV, NMODE)
        W2 = np.zeros((128, NG, 64), dtype=np.float32)
        # spect [B, alpha, j, m] -> xr/xi [j, mode, b]
        xr = spect[:, a_sl, :, :].transpose(2, 1, 3, 0).reshape(DV, NMODE, B)
        xi = (spect[:, 64 + s * AL:64 + (s + 1) * AL, :, :]
              .transpose(2, 1, 3, 0).reshape(DV, NMODE, B))
        X2 = np.empty((128, NG, 2, B), dtype=np.float32)
        for u in range(2):
            r0, r1_, r2_ = u * 64, u * 64 + 32, u * 64 + 64
            W2[r0:r1_, :, u * 32:(u + 1) * 32] = (
                Rr_t[:, :, u::2].transpose(0, 2, 1))
            W2[r1_:r2_, :, u * 32:(u + 1) * 32] = (
                Ri_t[:, :, u::2].transpose(0, 2, 1))
            X2[r0:r1_, :, 0, :] = xr[:, u::2, :]
            X2[r1_:r2_, :, 0, :] = -xi[:, u::2, :]
            X2[r0:r1_, :, 1, :] = xi[:, u::2, :]
            X2[r1_:r2_, :, 1, :] = xr[:, u::2, :]
        in2.append({"w2": W2.reshape(128, NG * 64).astype(NPBF16),
                    "x2": X2.reshape(128, NG * 8).astype(NPBF16)})
    r2 = run_bass_kernel_spmd(_get("p2a"), in2, list(range(NCORES)))
    LAST_EXEC_NS.append(r2.exec_time_ns)
    # y core result [128, (dg, 16)]: diagonal quadrants hold even/odd pair
    # groups -> reassemble [64=(u,i), (g, q, b)] -> [q, i, a_l, m, b]
    ys = []
    for s in range(NCORES):
        yr_ = r2.results[s]["y"].reshape(128, NG // 2, 16)
        yg = np.empty((64, NG, 2, B), dtype=yr_.dtype)
        yg[:, 0::2] = yr_[0:64, :, 0:8].reshape(64, NG // 2, 2, B)
        yg[:, 1::2] = yr_[64:128, :, 8:16].reshape(64, NG // 2, 2, B)
        t = yg.reshape(2, DV, NG, 2, B)                       # [u, i, g, q, b]
        t = t.transpose(3, 1, 2, 0, 4).reshape(2, DV, NMODE, B)
        ys.append(t.reshape(2, DV, AL, KY, B))
    yfull = np.stack(ys)                                       # [s, q, i, a_l, m, b]
    yfull = yfull.transpose(1, 2, 0, 3, 4, 5).reshape(2, DV, KX, KY, B)

    # ---------------- phase 2b ----------------
    in3 = []
    for c in range(NCORES):
        b, h = c // 2, c % 2
        ypk = np.zeros((128, 2816), dtype=NPBF16)
        # yb [(q, a), (i, m)]
        ybc = yfull[:, :, :, :, b].transpose(0, 2, 1, 3).reshape(128, DV * KY)
        ypk[:, 0:2048] = ybc.astype(NPBF16)
        ypk[:, 2048:2304] = _G[:, h * NXH:(h + 1) * NXH]
        ypk[0:64, 2304:2816] = _CY
        ypk[64:128, 2304:2816] = _CY
        in3.append({"ypk": ypk})
    r3 = run_bass_kernel_spmd(_get("p2b"), in3, list(range(NCORES)))
    LAST_EXEC_NS.append(r3.exec_time_ns)

    out = np.empty((B, NX, NY, DV), dtype=np.float32)
    for c in range(NCORES):
        b, h = c // 2, c % 2
        oh2 = r3.results[c]["oh2"].reshape(NXH, DV, NY)
        out[b, h * NXH:(h + 1) * NXH] = (
            oh2.transpose(0, 2, 1).astype(np.float32))
    return out
